# revision 1
# baseline (speedup 1.0000x reference)
"""AtomEncoderLayer kernel for 8 Trainium2 NeuronCores.

Sharding (per spec hint): data-parallel over batch (B=4) x node-half
(N=512 -> 2 halves of 256). Core c handles graph b=c//2, node rows
[h*256,(h+1)*256) with h=c%2. All edge_index gathers are per-graph
local; phase 1 (message passing + node update) needs only the full
*input* node features of its graph (replicated per pair, 512KB) plus
its own edge shard, so it runs with zero cross-core communication.
The edge update needs the *updated* nodes of the whole graph, so it
runs as phase 2 after a tiny (256KB/graph) host-side exchange of the
updated halves.

Hardcoded shapes: B=4, N=512, K=32, DIM=256, PDIM=128, MSG=128, H=16,
HD=64, EF=2.
"""

import numpy as np
import jax
import jax.numpy as jnp

LN_EPS = 1e-5
B, N, K = 4, 512, 32
DIM, PDIM, MSG = 256, 128, 128
H, HD = 16, 64
HALF = N // 2
NC = 8

_CACHE = {}


def _layernorm(x, g, b):
    mu = x.mean(-1, keepdims=True)
    var = ((x - mu) ** 2).mean(-1, keepdims=True)
    return (x - mu) * jax.lax.rsqrt(var + LN_EPS) * g + b


def _mlp(x, p):
    y = _layernorm(x, p['ln_g'], p['ln_b'])
    h = jax.nn.gelu(y @ p['w1'] + p['b1'], approximate=False)
    return h @ p['w2'] + p['b2']


def _phase1(node_g, node_own, edge_r, edge_i, edge_m, mask_own, params):
    """node_g: [N,DIM] full-graph input nodes; *_own/edge_*: this core's
    256-node shard. Returns updated node rows [HALF, DIM]."""
    embool = edge_m.astype(bool)
    emf = edge_m.astype(node_g.dtype)

    src = node_g @ params['node_src_w']            # [N, MSG]
    msg = edge_r @ params['edge_msg_w']            # [HALF, K, MSG]
    msg = msg + src[edge_i]                        # gather, per-graph local
    msg = msg + (node_own @ params['node_tgt_w'])[:, None, :]
    msg = _mlp(msg, params['msg_mlp'])
    msg = jnp.where(embool[..., None], msg, 0.0)

    # global (mean-pooled, gated) branch
    o = (msg * emf[..., None]).sum(-2) / (emf.sum(-1, keepdims=True) + 1e-6)
    o = jax.nn.sigmoid(node_own @ params['gating_w'] + params['gating_b']) * o
    dh = o @ params['out_w']

    # local GAT branch
    bias = jax.nn.leaky_relu(msg @ params['msg_lin_w'], negative_slope=0.01)
    bias = bias @ params['attn_bias_w']            # [HALF, K, H]
    bias = jnp.transpose(bias, (2, 0, 1))          # [H, HALF, K]
    neg = -jnp.finfo(bias.dtype).max
    bias = jnp.where(embool[None], bias, neg)
    attn = jax.nn.softmax(bias, axis=-1)           # [H, HALF, K]
    v = (msg @ params['gat_value_w']).reshape(HALF, K, H, HD)
    v = jnp.transpose(v, (2, 0, 1, 3))             # [H, HALF, K, HD]
    o2 = jnp.einsum('hnk,hnkd->hnd', attn, v)
    o2 = jnp.transpose(o2, (1, 0, 2)).reshape(HALF, H * HD)
    o2 = jax.nn.sigmoid(node_own @ params['gat_gating_w']
                        + params['gat_gating_b']) * o2
    dh = dh + o2 @ params['gat_out_w']

    node = node_own + dh
    node = node + _mlp(node, params['node_mlp'])
    node = jnp.where(mask_own.astype(bool)[..., None], node, 0.0)
    return node


def _phase2(node_full, node_own, edge_r, edge_i, params):
    """node_full: [N,DIM] updated nodes of the graph. Returns this
    core's updated edge shard [HALF, K, PDIM]."""
    s = node_full @ params['out_src_w']            # [N, PDIM]
    emsg = edge_r + s[edge_i]
    emsg = emsg + (node_own @ params['out_tgt_w'])[:, None, :]
    return edge_r + _mlp(emsg, params['edge_mlp'])


def _get_devices():
    devs = [d for d in jax.devices() if d.platform != 'cpu']
    if len(devs) < NC:
        devs = jax.devices()
    return devs[:NC]


def _tree_put(tree, dev):
    return jax.tree_util.tree_map(lambda x: jax.device_put(x, dev), tree)


def kernel(node_repr, edge_repr, edge_index, edge_mask, mask, params):
    node_repr = np.asarray(node_repr, dtype=np.float32)
    edge_repr = np.asarray(edge_repr, dtype=np.float32)
    idx32 = np.asarray(edge_index, dtype=np.int32)
    edge_mask = np.asarray(edge_mask, dtype=np.int32)
    mask = np.asarray(mask, dtype=np.int32)
    params = jax.tree_util.tree_map(
        lambda x: np.asarray(x, dtype=np.float32), dict(params))

    devs = _get_devices()
    if 'p1' not in _CACHE:
        _CACHE['p1'] = jax.jit(_phase1)
        _CACHE['p2'] = jax.jit(_phase2)
    p1, p2 = _CACHE['p1'], _CACHE['p2']

    params_d = [_tree_put(params, d) for d in devs]

    # --- phase 1: node update, 8-way (b, half) data parallel ---
    outs1 = []
    for c in range(NC):
        b, h = divmod(c, 2)
        sl = slice(h * HALF, (h + 1) * HALF)
        d = devs[c]
        args = (
            jax.device_put(node_repr[b], d),
            jax.device_put(node_repr[b, sl], d),
            jax.device_put(edge_repr[b, sl], d),
            jax.device_put(idx32[b, sl], d),
            jax.device_put(edge_mask[b, sl], d),
            jax.device_put(mask[b, sl], d),
            params_d[c],
        )
        outs1.append(p1(*args))
    node_new = np.empty((B, N, DIM), dtype=np.float32)
    for c in range(NC):
        b, h = divmod(c, 2)
        node_new[b, h * HALF:(h + 1) * HALF] = np.asarray(outs1[c])

    # --- phase 2: edge update ---
    outs2 = []
    for c in range(NC):
        b, h = divmod(c, 2)
        sl = slice(h * HALF, (h + 1) * HALF)
        d = devs[c]
        args = (
            jax.device_put(node_new[b], d),
            jax.device_put(node_new[b, sl], d),
            jax.device_put(edge_repr[b, sl], d),
            jax.device_put(idx32[b, sl], d),
            params_d[c],
        )
        outs2.append(p2(*args))
    edge_new = np.empty((B, N, K, PDIM), dtype=np.float32)
    for c in range(NC):
        b, h = divmod(c, 2)
        edge_new[b, h * HALF:(h + 1) * HALF] = np.asarray(outs2[c])

    return node_new, edge_new


# revision 2
# speedup vs baseline: 4.5196x; 4.5196x over previous
"""AtomEncoderLayer kernel for 8 Trainium2 NeuronCores.

Sharding (per spec hint): data-parallel over batch (B=4) x node-half
(N=512 -> 2 halves of 256). Core c handles graph b=c//2, node rows
[h*256,(h+1)*256) with h=c%2. All edge_index gathers are per-graph
local; phase 1 (message passing + node update) needs only the full
*input* node features of its graph (replicated per pair) plus its own
edge shard, so it runs with zero cross-core communication. The edge
update needs the *updated* nodes of the whole graph, so it runs as
phase 2 after a tiny (256KB/graph) host-side exchange of the updated
halves. Each phase is one pmap launch across the 8 cores.

Hardcoded shapes: B=4, N=512, K=32, DIM=256, PDIM=128, MSG=128, H=16,
HD=64, EF=2.
"""

import numpy as np
import jax
import jax.numpy as jnp

LN_EPS = 1e-5
B, N, K = 4, 512, 32
DIM, PDIM, MSG = 256, 128, 128
H, HD = 16, 64
HALF = N // 2
NC = 8

_CACHE = {}


def _layernorm(x, g, b):
    mu = x.mean(-1, keepdims=True)
    var = ((x - mu) ** 2).mean(-1, keepdims=True)
    return (x - mu) * jax.lax.rsqrt(var + LN_EPS) * g + b


def _mlp(x, p):
    y = _layernorm(x, p['ln_g'], p['ln_b'])
    h = jax.nn.gelu(y @ p['w1'] + p['b1'], approximate=False)
    return h @ p['w2'] + p['b2']


def _phase1(node_g, node_own, edge_r, edge_i, edge_m, mask_own, params):
    """node_g: [N,DIM] full-graph input nodes; *_own/edge_*: this core's
    256-node shard. Returns updated node rows [HALF, DIM]."""
    embool = edge_m.astype(bool)
    emf = edge_m.astype(node_g.dtype)

    src = node_g @ params['node_src_w']            # [N, MSG]
    msg = edge_r @ params['edge_msg_w']            # [HALF, K, MSG]
    msg = msg + src[edge_i]                        # gather, per-graph local
    msg = msg + (node_own @ params['node_tgt_w'])[:, None, :]
    msg = _mlp(msg, params['msg_mlp'])
    msg = jnp.where(embool[..., None], msg, 0.0)

    # global (mean-pooled, gated) branch
    o = (msg * emf[..., None]).sum(-2) / (emf.sum(-1, keepdims=True) + 1e-6)
    o = jax.nn.sigmoid(node_own @ params['gating_w'] + params['gating_b']) * o
    dh = o @ params['out_w']

    # local GAT branch
    bias = jax.nn.leaky_relu(msg @ params['msg_lin_w'], negative_slope=0.01)
    bias = bias @ params['attn_bias_w']            # [HALF, K, H]
    bias = jnp.transpose(bias, (2, 0, 1))          # [H, HALF, K]
    neg = -jnp.finfo(bias.dtype).max
    bias = jnp.where(embool[None], bias, neg)
    attn = jax.nn.softmax(bias, axis=-1)           # [H, HALF, K]
    v = (msg @ params['gat_value_w']).reshape(HALF, K, H, HD)
    v = jnp.transpose(v, (2, 0, 1, 3))             # [H, HALF, K, HD]
    o2 = jnp.einsum('hnk,hnkd->hnd', attn, v)
    o2 = jnp.transpose(o2, (1, 0, 2)).reshape(HALF, H * HD)
    o2 = jax.nn.sigmoid(node_own @ params['gat_gating_w']
                        + params['gat_gating_b']) * o2
    dh = dh + o2 @ params['gat_out_w']

    node = node_own + dh
    node = node + _mlp(node, params['node_mlp'])
    node = jnp.where(mask_own.astype(bool)[..., None], node, 0.0)
    return node


def _phase2(node_full, edge_r, edge_i, params):
    """node_full: [N,DIM] updated nodes of the graph; edge shard is the
    same one phase 1 used (stays on device). Returns [HALF, K, PDIM]."""
    s = node_full @ params['out_src_w']            # [N, PDIM]
    node_own = jax.lax.dynamic_slice_in_dim(
        node_full, jax.lax.axis_index('c') % 2 * HALF, HALF, 0)
    emsg = edge_r + s[edge_i]
    emsg = emsg + (node_own @ params['out_tgt_w'])[:, None, :]
    return edge_r + _mlp(emsg, params['edge_mlp'])


def _get_devices():
    devs = [d for d in jax.devices() if d.platform != 'cpu']
    if len(devs) < NC:
        devs = jax.devices()
    return devs[:NC]


def kernel(node_repr, edge_repr, edge_index, edge_mask, mask, params):
    node_repr = np.asarray(node_repr, dtype=np.float32)
    edge_repr = np.asarray(edge_repr, dtype=np.float32)
    idx32 = np.asarray(edge_index, dtype=np.int32)
    edge_mask = np.asarray(edge_mask, dtype=np.int32)
    mask = np.asarray(mask, dtype=np.int32)
    params = jax.tree_util.tree_map(
        lambda x: np.asarray(x, dtype=np.float32), dict(params))

    devs = _get_devices()
    if 'p1' not in _CACHE:
        _CACHE['p1'] = jax.pmap(
            _phase1, axis_name='c', devices=devs,
            in_axes=(0, 0, 0, 0, 0, 0, None))
        _CACHE['p2'] = jax.pmap(
            _phase2, axis_name='c', devices=devs,
            in_axes=(0, 0, 0, None))
    p1, p2 = _CACHE['p1'], _CACHE['p2']

    # stack per-core shards: core c = (b=c//2, h=c%2)
    def shard(x):  # [B, N, ...] -> [NC, HALF, ...]
        return x.reshape(B * 2, HALF, *x.shape[2:])

    node_g_st = np.repeat(node_repr, 2, axis=0)        # [NC, N, DIM]
    node_own_st = shard(node_repr)
    edge_r_st = shard(edge_repr)
    idx_st = shard(idx32)
    em_st = shard(edge_mask)
    m_st = shard(mask)

    # keep edge shards on device for phase 2
    edge_r_d = jax.device_put_sharded(list(edge_r_st), devs)
    idx_d = jax.device_put_sharded(list(idx_st), devs)

    out1 = p1(node_g_st, node_own_st, edge_r_d, idx_d, em_st, m_st, params)
    node_new = np.asarray(out1).reshape(B, N, DIM)

    node_full_st = np.repeat(node_new, 2, axis=0)      # [NC, N, DIM]
    out2 = p2(node_full_st, edge_r_d, idx_d, params)
    edge_new = np.asarray(out2).reshape(B, N, K, PDIM)

    return node_new, edge_new
